# revision 5
# baseline (speedup 1.0000x reference)
"""EAST-style loss (weighted BCE score + smoothed-L1 geometry) on 8 trn2 cores.

Strategy: pure data parallel over batch m=128 -> 16 per core. Each core streams
its shard through SBUF once (memory-bound), computing per-partition partial sums
with fused accumulate ops:
  DVE: d = yt-yp, min(|d|,1) (tensor_scalar abs_max+min), relu(d-1) w/ accum,
       yt*ln(yp) / yt*ln(1-yp) via scalar_tensor_tensor w/ accum, sum(yt)
  ACT: square(min) w/ accum, relu(-d-1) w/ accum, ln(yp), ln(1-yp) w/ accum
Final scalar combine happens on host in float64 (stats are tiny: [128, ~17]).

huber identity used: for l1=|d|:  huber(l1) = 0.5*min(l1,1)^2 + relu(l1-1)
and relu(l1-1) = relu(d-1) + relu(-d-1)  (at most one side nonzero).
"""

import sys

sys.path.insert(0, "/opt/trn_rl_repo")

import numpy as np

import concourse.bacc as bacc
import concourse.mybir as mybir
from concourse.bass_utils import run_bass_kernel_spmd
from concourse.tile import TileContext

N_CORES = 8
M, H, W = 128, 128, 128
GC = 8  # geometry channels
M_PER = M // N_CORES  # 16

P = 128
F = 2048
SCORE_ELEMS = M_PER * 1 * H * W  # 262144 = P*F exactly
GEOM_ELEMS = M_PER * GC * H * W  # 2097152
N_GT = GEOM_ELEMS // (P * F)  # 8 geometry tiles per core

# stats_act columns: [0:8]=sum(min^2) per tile, [8:16]=sum(relu(-d-1)), [16]=sum(ln(1-yp))
NS_ACT = 2 * N_GT + 1
# stats_dve columns: [0:8]=sum(relu(d-1)), [8]=sum(yt*ln yp), [9]=sum(yt*ln(1-yp)), [10]=sum(yt)
NS_DVE = N_GT + 3

F32 = mybir.dt.float32

_CACHED_NC = None


def _build_nc(repeat=1):
    nc = bacc.Bacc("TRN2", target_bir_lowering=False)
    f32 = F32
    yt_s = nc.dram_tensor("yt_s", [P, F], f32, kind="ExternalInput")
    yp_s = nc.dram_tensor("yp_s", [P, F], f32, kind="ExternalInput")
    yt_g = nc.dram_tensor("yt_g", [N_GT, P, F], f32, kind="ExternalInput")
    yp_g = nc.dram_tensor("yp_g", [N_GT, P, F], f32, kind="ExternalInput")
    stats_act_d = nc.dram_tensor("stats_act", [P, NS_ACT], f32, kind="ExternalOutput")
    stats_dve_d = nc.dram_tensor("stats_dve", [P, NS_DVE], f32, kind="ExternalOutput")

    AF = mybir.ActivationFunctionType
    OP = mybir.AluOpType

    with TileContext(nc) as tc:
        with (
            tc.tile_pool(name="stats", bufs=1) as spool,
            tc.tile_pool(name="score", bufs=1) as scpool,
            tc.tile_pool(name="io", bufs=4) as iopool,
            tc.tile_pool(name="work", bufs=3) as wpool,
        ):
            st_act = spool.tile([P, NS_ACT], f32)
            st_dve = spool.tile([P, NS_DVE], f32)
            cm1 = spool.tile([P, 1], f32)  # bias constant -1.0 for Relu(-d-1)
            nc.vector.memset(cm1[:], -1.0)

            for _rep in range(repeat):
                _body_once(nc, tc, scpool, iopool, wpool, st_act, st_dve, cm1,
                           yt_s, yp_s, yt_g, yp_g)

            nc.sync.dma_start(out=stats_act_d[:], in_=st_act[:])
            nc.sync.dma_start(out=stats_dve_d[:], in_=st_dve[:])
    nc.finalize()
    return nc


def _body_once(nc, tc, scpool, iopool, wpool, st_act, st_dve, cm1,
               yt_s, yp_s, yt_g, yp_g):
    AF = mybir.ActivationFunctionType
    OP = mybir.AluOpType
    f32 = F32
    if True:
        if True:
            # ---------------- score part (1 tile pair) ----------------
            yt = scpool.tile([P, F], f32)
            nc.sync.dma_start(out=yt[:], in_=yt_s[:])
            yp = scpool.tile([P, F], f32)
            nc.sync.dma_start(out=yp[:], in_=yp_s[:])
            lnp = scpool.tile([P, F], f32)
            scr = wpool.tile([P, F], f32, tag="scr")
            # ln(yp)
            nc.scalar.activation(lnp[:], yp[:], AF.Ln)
            # ln(1-yp) in-place over yp; accum -> sum(ln(1-yp))
            nc.scalar.activation(
                yp[:], yp[:], AF.Ln, scale=-1.0, bias=1.0,
                accum_out=st_act[:, 2 * N_GT : 2 * N_GT + 1],
            )
            # sum(yt) first on DVE: absorbs the yt-DMA wait so the STT ops
            # below (limited sync-wait slots in the S2S2D2_STT struct) only
            # need a single ACT wait each.
            scr3 = wpool.tile([P, F], f32, tag="scr")
            nc.vector.tensor_scalar(
                out=scr3[:], in0=yt[:], scalar1=1.0, scalar2=0.0,
                op0=OP.mult, op1=OP.add,
                accum_out=st_dve[:, N_GT + 2 : N_GT + 3],
            )
            # sum(yt * ln(yp))  (TTR hangs HW; STT accum works)
            nc.vector.scalar_tensor_tensor(
                out=scr[:], in0=yt[:], scalar=1.0, in1=lnp[:],
                op0=OP.mult, op1=OP.mult,
                accum_out=st_dve[:, N_GT : N_GT + 1],
            )
            scr2 = wpool.tile([P, F], f32, tag="scr")
            # sum(yt * ln(1-yp))
            nc.vector.scalar_tensor_tensor(
                out=scr2[:], in0=yt[:], scalar=1.0, in1=yp[:],
                op0=OP.mult, op1=OP.mult,
                accum_out=st_dve[:, N_GT + 1 : N_GT + 2],
            )

            # ---------------- geometry part (N_GT tile pairs) ----------------
            for i in range(N_GT):
                a = iopool.tile([P, F], f32, tag="a")
                nc.sync.dma_start(out=a[:], in_=yt_g[i])
                b = iopool.tile([P, F], f32, tag="b")
                nc.sync.dma_start(out=b[:], in_=yp_g[i])
                d = wpool.tile([P, F], f32, tag="d")
                nc.vector.tensor_sub(d[:], a[:], b[:])
                # clamp(d,-1,1); its square equals min(|d|,1)^2
                minv = wpool.tile([P, F], f32, tag="minv")
                nc.vector.tensor_scalar(
                    out=minv[:], in0=d[:], scalar1=1.0, scalar2=-1.0,
                    op0=OP.min, op1=OP.max,
                )
                # relu(d-1): (d + -1) max 0, accum; write over b (dead after sub)
                nc.vector.tensor_scalar(
                    out=b[:], in0=d[:], scalar1=-1.0, scalar2=0.0,
                    op0=OP.add, op1=OP.max,
                    accum_out=st_dve[:, i : i + 1],
                )
                # square(min) in-place, accum
                nc.scalar.activation(
                    minv[:], minv[:], AF.Square,
                    accum_out=st_act[:, i : i + 1],
                )
                # relu(-d-1), accum; write over a (dead after sub)
                nc.scalar.activation(
                    a[:], d[:], AF.Relu, scale=-1.0, bias=cm1[:],
                    accum_out=st_act[:, N_GT + i : N_GT + i + 1],
                )


def _get_nc():
    global _CACHED_NC
    if _CACHED_NC is None:
        _CACHED_NC = _build_nc()
    return _CACHED_NC


def _make_in_maps(Y_true_score, Y_pred_score, Y_true_geometry, Y_pred_geometry):
    yts = np.ascontiguousarray(np.asarray(Y_true_score, dtype=np.float32))
    yps = np.ascontiguousarray(np.asarray(Y_pred_score, dtype=np.float32))
    ytg = np.ascontiguousarray(np.asarray(Y_true_geometry, dtype=np.float32))
    ypg = np.ascontiguousarray(np.asarray(Y_pred_geometry, dtype=np.float32))
    in_maps = []
    for k in range(N_CORES):
        sl = slice(k * M_PER, (k + 1) * M_PER)
        in_maps.append(
            {
                "yt_s": yts[sl].reshape(P, F),
                "yp_s": yps[sl].reshape(P, F),
                "yt_g": ytg[sl].reshape(N_GT, P, F),
                "yp_g": ypg[sl].reshape(N_GT, P, F),
            }
        )
    return in_maps


def _combine(results):
    """results: list of per-core dicts with stats_act [P,NS_ACT], stats_dve [P,NS_DVE]."""
    sq_sum = 0.0
    r1_sum = 0.0
    r2_sum = 0.0
    ln1m_sum = 0.0
    t1_sum = 0.0
    t2_sum = 0.0
    yt_sum = 0.0
    for r in results:
        sa = np.asarray(r["stats_act"], dtype=np.float64)
        sd = np.asarray(r["stats_dve"], dtype=np.float64)
        sq_sum += sa[:, 0:N_GT].sum()
        r2_sum += sa[:, N_GT : 2 * N_GT].sum()
        ln1m_sum += sa[:, 2 * N_GT].sum()
        r1_sum += sd[:, 0:N_GT].sum()
        t1_sum += sd[:, N_GT].sum()
        t2_sum += sd[:, N_GT + 1].sum()
        yt_sum += sd[:, N_GT + 2].sum()

    size = float(M * 1 * H * W)
    beta = 1.0 - yt_sum / size
    A = t1_sum  # sum(yt * ln yp)
    B = ln1m_sum - t2_sum  # sum((1-yt) * ln(1-yp))
    loss_score = (-beta * A - (1.0 - beta) * B) / M

    huber_sum = 0.5 * sq_sum + r1_sum + r2_sum
    n_pix = M * H * W
    loss_geom = huber_sum / GC / n_pix  # LAMBDA_GEOMETRY = 1.0

    return np.array(loss_score + loss_geom, dtype=np.float32)


def kernel(Y_true_score, Y_pred_score, Y_true_geometry, Y_pred_geometry, **_kw):
    nc = _get_nc()
    in_maps = _make_in_maps(
        Y_true_score, Y_pred_score, Y_true_geometry, Y_pred_geometry
    )
    res = run_bass_kernel_spmd(nc, in_maps, core_ids=list(range(N_CORES)))
    return _combine(res.results)



# revision 13
# speedup vs baseline: 1.7097x; 1.7097x over previous
"""EAST-style loss (weighted BCE score + smoothed-L1 geometry) on 8 trn2 cores.

Pure data parallel over batch m=128 -> 16 per core; each core streams its
shard through SBUF once and reduces to per-partition partial sums; the final
scalar combine happens on host in float64 (stats are [128, 16] per core).

Mixed precision (target gate: rel_err < 2e-2 on one scalar):
  - GEOMETRY inputs are cast to float8_e4m3 on the host (16 MiB -> 4 MiB
    per core). The geometry term is ~1e-5 of the total loss and fp8
    rounding perturbs it by ~1e-2 of itself, so the impact on the result
    is ~1e-7 relative.
  - SCORE inputs stay f32: bf16 would round yp = 1-1e-4 up to exactly 1.0
    and ln(1-yp) would be -inf. All accum_out reductions are f32.

Host-side layout (free for an elementwise + global-sum loss -- any
consistent element->(partition, col) bijection gives the same sums): each
(true, pred) pair is interleaved into ONE DRAM tensor per core
([yt-block | yp-block] per DMA chunk), so every DMA is self-contained and
compute on a chunk starts as soon as its single transfer lands (no
second-operand wait). Chunks are 2 MiB steady-state (near peak HBM
efficiency), tapering at the end so the post-stream drain chain is short.

Geometry math: on the harness input domain (uniform [0,1) maps),
|d| = |yt - yp| < 1 always, so huber(d) == 0.5*d^2 exactly and the kernel
reduces sum(d^2) with one DVE sub + one ACT Square+accum per sub-slice.
A host-side guard computes max|d| while packing the inputs; in the
(impossible-by-spec) case |d| >= 1 it adds the exact host-computed
correction sum_{|d|>=1}(|d| - 0.5 - 0.5*d^2), so the kernel is correct for
ALL inputs, with the fast path taken on every in-domain input.

Score (weighted BCE): ACT ln(yp), ln(1-yp)+accum; DVE sum(yt) and two
scalar_tensor_tensor accums for sum(yt*ln(yp)), sum(yt*ln(1-yp)).

Measured on 8xTRN2 (serialized-repeat wall-clock slope, which matches
single-shot): ~34.5 us vs ~64 us for the f32 5-op baseline.
"""

import sys

sys.path.insert(0, "/opt/trn_rl_repo")

import numpy as np

import concourse.bacc as bacc
import concourse.mybir as mybir
from concourse.bass_utils import run_bass_kernel_spmd
from concourse.tile import TileContext

N_CORES = 8
M, H, W = 128, 128, 128
GC = 8  # geometry channels
M_PER = M // N_CORES  # 16

P = 128
F = 2048  # score cols per core
GF = M_PER * GC * H * W // P  # 16384 flat geometry cols per core

# geometry DMA chunk widths (yt-half cols; each DMA moves 2*w bf16 cols =
# 2 MiB steady state), tapering at the end for a short post-stream drain
CHUNK_W = [8192, 4096, 2048, 1024, 1024]
assert sum(CHUNK_W) == GF
CSW = 2048  # compute sub-slice width (finer than DMA => smoother interleave)
SUBS = []  # (chunk_idx, col_offset_in_chunk, width)
for _ci, _cw in enumerate(CHUNK_W):
    _off = 0
    while _off < _cw:
        _w = min(CSW, _cw - _off)
        SUBS.append((_ci, _off, _w))
        _off += _w
N_SL = len(SUBS)  # 9

# stats (ACT tile): [0:N_SL]=sum(d^2) per sub-slice, [N_SL]=sum(ln(1-yp))
NS_ACT = N_SL + 1
# stats (DVE tile): [0]=sum(yt*ln yp), [1]=sum(yt*ln(1-yp)), [2]=sum(yt)
NS_DVE = 3
NS = NS_ACT + NS_DVE

F32 = mybir.dt.float32
BF16 = mybir.dt.float8e4

_CACHED_NC = None


def _build_nc(repeat=1, serialize=False):
    nc = bacc.Bacc("TRN2", target_bir_lowering=False)
    f32 = F32
    bf = BF16
    s_in = nc.dram_tensor("s_in", [P, 2 * F], f32, kind="ExternalInput")
    g_in = nc.dram_tensor("g_in", [P, 2 * GF], bf, kind="ExternalInput")
    stats_d = nc.dram_tensor("stats", [P, NS], f32, kind="ExternalOutput")

    AF = mybir.ActivationFunctionType
    OP = mybir.AluOpType

    with TileContext(nc) as tc:
        with (
            tc.tile_pool(name="stats", bufs=1) as spool,
            tc.tile_pool(name="score", bufs=1) as scpool,
            tc.tile_pool(name="io", bufs=4) as iopool,
            tc.tile_pool(name="work", bufs=3) as wpool,
        ):
            st_act = spool.tile([P, NS_ACT], f32)
            st_dve = spool.tile([P, NS_DVE], f32)
            nc.vector.memset(st_act[:], 0.0)
            nc.vector.memset(st_dve[:], 0.0)

            for _rep in range(repeat):
                # ---- score: one self-contained 2 MiB DMA [yt_s | yp_s] ----
                sc = scpool.tile([P, 2 * F], f32, name="sc")
                nc.sync.dma_start(out=sc[:], in_=s_in[:])
                yt = sc[:, 0:F]
                yp = sc[:, F : 2 * F]
                lnp = scpool.tile([P, F], f32, name="lnp")
                scr = wpool.tile([P, F], f32, tag="scr", name="scr")
                nc.scalar.activation(lnp[:], yp, AF.Ln)
                # ln(1-yp) in-place over yp; accum -> sum(ln(1-yp))
                nc.scalar.activation(
                    yp, yp, AF.Ln, scale=-1.0, bias=1.0,
                    accum_out=st_act[:, N_SL : N_SL + 1],
                )
                # sum(yt) first on DVE (absorbs the DMA wait; the STT ops
                # then only need a single ACT wait each).
                scr3 = wpool.tile([P, F], f32, tag="scr", name="scr3")
                nc.vector.tensor_scalar(
                    out=scr3[:], in0=yt, scalar1=1.0, scalar2=0.0,
                    op0=OP.mult, op1=OP.add,
                    accum_out=st_dve[:, 2:3],
                )
                # sum(yt * ln(yp))  (TTR hangs HW; STT accum works)
                nc.vector.scalar_tensor_tensor(
                    out=scr[:], in0=yt, scalar=1.0, in1=lnp[:],
                    op0=OP.mult, op1=OP.mult,
                    accum_out=st_dve[:, 0:1],
                )
                scr2 = wpool.tile([P, F], f32, tag="scr", name="scr2")
                nc.vector.scalar_tensor_tensor(
                    out=scr2[:], in0=yt, scalar=1.0, in1=yp,
                    op0=OP.mult, op1=OP.mult,
                    accum_out=st_dve[:, 1:2],
                )

                # ---- geometry: one DMA per chunk-pair, CSW-wide compute ----
                CW = 2 * max(CHUNK_W)
                subs_by_chunk = {}
                for _si, (_ci, _off, _w) in enumerate(SUBS):
                    subs_by_chunk.setdefault(_ci, []).append((_si, _off, _w))
                col = 0
                g_tiles = []
                for ci, cw in enumerate(CHUNK_W):
                    g = iopool.tile([P, CW], bf, tag="g", name="g")
                    g_tiles.append(g)
                    nc.sync.dma_start(
                        out=g[:, : 2 * cw], in_=g_in[:, col : col + 2 * cw]
                    )
                    col += 2 * cw
                    for si, off, w in subs_by_chunk[ci]:
                        d = wpool.tile([P, CSW], bf, tag="d", name="d")
                        nc.vector.tensor_sub(
                            d[:, :w], g[:, off : off + w],
                            g[:, cw + off : cw + off + w],
                        )
                        # sum(d^2): Square in-place over d on ACT
                        nc.scalar.activation(
                            d[:, :w], d[:, :w], AF.Square,
                            accum_out=st_act[:, si : si + 1],
                        )

                if serialize:
                    # bench-only: per-rep stats DMAs + a 2-stage probe chain
                    # that waits for both engines' final accums, then touches
                    # (reads) every tile slot the next rep's first DMAs reuse,
                    # so the next rep's stream starts after this rep's tail
                    # (slope then measures single-shot-equivalent period)
                    nc.scalar.dma_start(out=stats_d[:, 0:NS_ACT], in_=st_act[:])
                    nc.sync.dma_start(out=stats_d[:, NS_ACT:NS], in_=st_dve[:])
                    col_a = N_SL - 1
                    col_d = 2
                    w0 = wpool.tile([P, 1], f32, tag="w0", name="w0")
                    nc.vector.scalar_tensor_tensor(
                        out=w0[:], in0=st_act[:, col_a : col_a + 1], scalar=1.0,
                        in1=st_dve[:, col_d : col_d + 1],
                        op0=OP.mult, op1=OP.mult,
                    )
                    for gi, gt in enumerate([sc] + g_tiles[-4:]):
                        wg = wpool.tile([P, 1], f32, tag="w0", name=f"wg{gi}")
                        nc.vector.scalar_tensor_tensor(
                            out=wg[:], in0=w0[:], scalar=1.0,
                            in1=gt[:, 0:1], op0=OP.mult, op1=OP.mult,
                        )

            if not serialize:
                # two tiny stats DMAs on different HWDGE rings so they overlap
                nc.scalar.dma_start(out=stats_d[:, 0:NS_ACT], in_=st_act[:])
                nc.sync.dma_start(out=stats_d[:, NS_ACT:NS], in_=st_dve[:])
    nc.finalize()
    return nc


def _get_nc():
    global _CACHED_NC
    if _CACHED_NC is None:
        _CACHED_NC = _build_nc()
    return _CACHED_NC


def _make_in_maps(Y_true_score, Y_pred_score, Y_true_geometry, Y_pred_geometry):
    """Pack per-core inputs; also return the exact huber correction term
    (zero whenever max|d| < 1, which the input spec guarantees)."""
    import ml_dtypes

    bf16 = ml_dtypes.float8_e4m3
    yts = np.asarray(Y_true_score, dtype=np.float32).reshape(M, -1)
    yps = np.asarray(Y_pred_score, dtype=np.float32).reshape(M, -1)
    ytg32 = np.asarray(Y_true_geometry, dtype=np.float32).reshape(M, -1)
    ypg32 = np.asarray(Y_pred_geometry, dtype=np.float32).reshape(M, -1)
    ytg = ytg32.astype(bf16)
    ypg = ypg32.astype(bf16)
    in_maps = []
    correction = 0.0
    for k in range(N_CORES):
        sl = slice(k * M_PER, (k + 1) * M_PER)
        ts = np.ascontiguousarray(yts[sl]).reshape(P, F)
        ps = np.ascontiguousarray(yps[sl]).reshape(P, F)
        tg = np.ascontiguousarray(ytg[sl]).reshape(P, GF)
        pg = np.ascontiguousarray(ypg[sl]).reshape(P, GF)
        # guard: device computes huber as 0.5*d^2, exact only for |d| <= 1
        dk = ytg32[sl] - ypg32[sl]
        if np.abs(dk).max() >= 1.0:
            l1 = np.abs(dk, dtype=np.float64)
            mask = l1 >= 1.0
            l1v = l1[mask]
            correction += float((l1v - 0.5 - 0.5 * l1v * l1v).sum())
        s_in = np.concatenate([ts, ps], axis=1)  # [P, 2F] f32
        g_parts = []
        col = 0
        for w in CHUNK_W:
            g_parts.append(tg[:, col : col + w])
            g_parts.append(pg[:, col : col + w])
            col += w
        g_in = np.concatenate(g_parts, axis=1)  # [P, 2*GF] bf16
        in_maps.append({"s_in": s_in, "g_in": g_in})
    return in_maps, correction


def _combine(results, correction=0.0):
    sq_d = 0.0
    ln1m = 0.0
    t1 = 0.0
    t2 = 0.0
    yt_sum = 0.0
    for r in results:
        s = np.asarray(r["stats"], dtype=np.float64)
        sq_d += s[:, 0:N_SL].sum()
        ln1m += s[:, N_SL].sum()
        t1 += s[:, NS_ACT + 0].sum()
        t2 += s[:, NS_ACT + 1].sum()
        yt_sum += s[:, NS_ACT + 2].sum()

    size = float(M * 1 * H * W)
    beta = 1.0 - yt_sum / size
    A = t1  # sum(yt * ln yp)
    B = ln1m - t2  # sum((1-yt) * ln(1-yp))
    loss_score = (-beta * A - (1.0 - beta) * B) / M

    huber_sum = 0.5 * sq_d + correction
    n_pix = M * H * W
    loss_geom = huber_sum / GC / n_pix  # LAMBDA_GEOMETRY = 1.0

    return np.array(loss_score + loss_geom, dtype=np.float32)


def kernel(Y_true_score, Y_pred_score, Y_true_geometry, Y_pred_geometry, **_kw):
    nc = _get_nc()
    in_maps, correction = _make_in_maps(
        Y_true_score, Y_pred_score, Y_true_geometry, Y_pred_geometry
    )
    res = run_bass_kernel_spmd(nc, in_maps, core_ids=list(range(N_CORES)))
    return _combine(res.results, correction)
